# revision 1
# baseline (speedup 1.0000x reference)
"""TRN2 Bass kernel for nn_Attention_15590731285136.

Computation (per batch b):
    g      = diag(W) * K[b]                       # [d]
    score  = relu(V[b] @ (g[:,None]*w1) + b1) @ w2 + b2   # [h]
    score  = where(mask[b], MASK_FILL, score)
    alpha  = softmax(score)                        # over h
    out[b] = alpha @ V[b]                          # [d]

Sharding: data-parallel over batch, 8 batches per core on 8 NeuronCores.

Key transformations:
  * The elementwise gate folds into the weight matrix: V*g @ w1 = V @ (g[:,None]*w1).
  * w2 folds into w1's columns by |w2| with a sign-grouping permutation, so the
    w2-dot becomes two plain row-sums of the relu output; those are computed
    for free by the fused relu+accumulate paths on ScalarE (ACT) and VectorE.
  * V is pre-cast to fp16 on the host; the d-major (transposed) copy needed for
    the fc1 contraction is produced by the DMA xbar transpose during the load,
    so the PE runs only the essential matmuls.
  * softmax skips max-subtraction (scores are O(0.1); masked entries get an
    additive -2^32 bias so exp underflows to exactly 0); normalization happens
    once at the end on the [1, 512] pooled accumulator.
"""

import numpy as np

B, H, D, HID = 64, 2048, 512, 512
NCORES = 8
BPC = B // NCORES          # batches per core
HT = H // 128              # 16 h-tiles per batch
DC = D // 128              # 4 contraction chunks
MASK_FILL = -2.0**32 + 1.0


def _build(hp, b2val, has_bias):
    import concourse.mybir as mybir
    from concourse import bacc
    from concourse.tile import TileContext
    from concourse.masks import make_identity

    F32 = mybir.dt.float32
    F16 = mybir.dt.float16
    ACTF = mybir.ActivationFunctionType
    ALU = mybir.AluOpType

    nc = bacc.Bacc(trn_type="TRN2", num_devices=NCORES)

    VB = nc.dram_tensor("VB", (BPC, H, D), F16, kind="ExternalInput")
    GT = nc.dram_tensor("GT", (BPC, 128, DC), F32, kind="ExternalInput")
    MB = nc.dram_tensor("MB", (BPC, 128, HT), F32, kind="ExternalInput")
    WA = nc.dram_tensor("WA", (D, HID), F32, kind="ExternalInput")
    if has_bias:
        BI = nc.dram_tensor("BI", (1, HID), F32, kind="ExternalInput")
    OUT = nc.dram_tensor("OUT", (BPC, D), F32, kind="ExternalOutput")

    with TileContext(nc) as tc:
        with (
            tc.tile_pool(name="const", bufs=1) as cpool,
            tc.tile_pool(name="v", bufs=3) as vpool,
            tc.tile_pool(name="vt", bufs=4 * DC) as vtpool,
            tc.tile_pool(name="w12", bufs=2) as wpool,
            tc.tile_pool(name="small", bufs=2) as spool,
            tc.tile_pool(name="scr", bufs=2) as scrpool,
            tc.tile_pool(name="fin", bufs=2) as finpool,
            tc.tile_pool(name="fc1_ps", bufs=2, space="PSUM") as fc1ps,
            tc.tile_pool(name="vt_ps", bufs=2, space="PSUM") as vtps,
            tc.tile_pool(name="tot_ps", bufs=2, space="PSUM") as totps,
            tc.tile_pool(name="acc_ps", bufs=2, space="PSUM") as accps,
        ):
            # ---- one-time constants ----
            ones_col = cpool.tile([128, 1], F16, tag="ones")
            nc.vector.memset(ones_col, 1.0)
            ident = cpool.tile([128, 128], F16, tag="ident")
            make_identity(nc, ident)

            # WA as [128, DC*HID]: chunk c at cols [c*HID, (c+1)*HID)
            wabs = cpool.tile([128, DC * HID], F32, tag="wabs")
            nc.sync.dma_start(
                out=wabs.rearrange("p (c n) -> p c n", c=DC),
                in_=WA.ap().rearrange("(c p) n -> p c n", p=128),
            )
            if has_bias:
                ones_row = cpool.tile([1, 128], F16, tag="orr")
                nc.vector.memset(ones_row, 1.0)
                bias_sb = cpool.tile([1, HID], F16, tag="bias")
                bias_f = cpool.tile([1, HID], F32, tag="biasf")
                nc.sync.dma_start(out=bias_f, in_=BI.ap())
                nc.vector.tensor_copy(bias_sb, bias_f)

            # ---- all batches' gate columns and mask biases in two DMAs ----
            gall = cpool.tile([128, BPC * DC], F32, tag="gall")
            nc.sync.dma_start(
                out=gall.rearrange("p (b c) -> p b c", b=BPC),
                in_=GT.ap().rearrange("b p c -> p b c"),
            )
            mall = cpool.tile([128, BPC * HT], F32, tag="mall")
            nc.sync.dma_start(
                out=mall.rearrange("p (b j) -> p b j", b=BPC),
                in_=MB.ap().rearrange("b p j -> p b j"),
            )
            # one staging tile for all 8 outputs; single store at the end
            oball = cpool.tile([1, BPC * D], F32, tag="oball")

            PRE = 3   # batches of V-loads in flight ahead of compute
            RAMP = 2  # leading batches whose V^T comes from PE transposes

            def emit_loads(bi):
                if bi >= RAMP:
                    vts = []
                    for c in range(DC):
                        vt = vtpool.tile([128, H], F16, tag="vt")
                        nc.sync.dma_start(
                            out=vt,
                            in_=VB.ap()[bi, :, c * 128:(c + 1) * 128],
                            transpose=True,
                        )
                        vts.append(vt)
                else:
                    vts = None
                v_all = vpool.tile([128, HT * D], F16, tag="v")
                v3 = v_all.rearrange("p (j d) -> p j d", j=HT)
                for q in range(4):
                    nc.gpsimd.dma_start(
                        out=v3[:, 4 * q:4 * q + 4, :],
                        in_=VB.ap()[bi, 512 * q:512 * (q + 1), :]
                            .rearrange("(j p) d -> p j d", p=128),
                    )
                return vts, v3

            pending = [emit_loads(bi) for bi in range(min(PRE, BPC))]

            for bi in range(BPC):
                if bi + PRE < BPC:
                    pending.append(emit_loads(bi + PRE))
                vts, v3 = pending.pop(0)
                if vts is None:
                    # ramp batch: transpose on the PE from the natural tiles,
                    # 128x128 blocks into fp16 PSUM, copy back on ACT/DVE
                    vts = []
                    for _c in range(DC):
                        vt_r = vtpool.tile([128, H], F16, tag="vt")
                        vts.append(vt_r)
                    for c in range(DC):
                        for grp in range(4):
                            tp = vtps.tile([128, 512], F16, tag="vtp")
                            for t in range(4):
                                j = grp * 4 + t
                                nc.tensor.transpose(
                                    tp[:, t * 128:(t + 1) * 128],
                                    v3[:, j, c * 128:(c + 1) * 128],
                                    ident,
                                )
                            eng = nc.scalar if (c + grp) % 2 == 0 else nc.vector
                            cb = eng.tensor_copy if eng is nc.vector else eng.copy
                            cb(vts[c][:, grp * 512:(grp + 1) * 512], tp)
                vts = list(vts)
                gcol = gall[:, bi * DC:(bi + 1) * DC]
                mb = mall[:, bi * HT:(bi + 1) * HT]

                # ---- gate the packed weights: W12[d, :] = g[d] * Wabs[d, :] ----
                w12 = wpool.tile([128, DC * HID], F16, tag="w12")
                for c in range(DC):
                    nc.vector.tensor_scalar_mul(
                        w12[:, c * HID:(c + 1) * HID],
                        wabs[:, c * HID:(c + 1) * HID],
                        gcol[:, c:c + 1],
                    )

                sp = spool.tile([128, HT], F32, tag="sp")
                sn = spool.tile([128, HT], F32, tag="sn")
                if hp == 0:
                    nc.vector.memset(sp, 0.0)
                if hp == HID:
                    nc.vector.memset(sn, 0.0)

                # ---- fc1 + fused relu/rowsum per tok-tile ----
                for j in range(HT):
                    fc1 = fc1ps.tile([128, HID], F32, tag="fc1")
                    for c in range(DC):
                        nc.tensor.matmul(
                            out=fc1,
                            lhsT=vts[c][:, j * 128:(j + 1) * 128],
                            rhs=w12[:, c * HID:(c + 1) * HID],
                            start=(c == 0),
                            stop=(c == DC - 1) and not has_bias,
                        )
                    if has_bias:
                        nc.tensor.matmul(
                            out=fc1, lhsT=ones_row, rhs=bias_sb,
                            start=False, stop=True,
                        )
                    # positive-w2 half on ACT (fused relu+rowsum)...
                    if hp > 0:
                        scra = scrpool.tile([128, HID], F16, tag="scra")
                        nc.scalar.activation(
                            out=scra[:, :hp], in_=fc1[:, :hp], func=ACTF.Relu,
                            accum_out=sp[:, j:j + 1],
                        )
                    # ...negative-w2 half on DVE (max(x,0) + add-reduce)
                    if hp < HID:
                        scrd = scrpool.tile([128, HID], F16, tag="scrd")
                        nc.vector.tensor_scalar(
                            out=scrd[:, hp:], in0=fc1[:, hp:],
                            scalar1=0.0, scalar2=None,
                            op0=ALU.max, op1=ALU.add,
                            accum_out=sn[:, j:j + 1],
                        )

                # ---- scores -> masked -> exp ----
                sc = spool.tile([128, HT], F32, tag="sc")
                nc.vector.tensor_sub(sc, sp, sn)
                scm = spool.tile([128, HT], F32, tag="scm")
                nc.vector.tensor_add(scm, sc, mb)
                alpha = spool.tile([128, HT], F16, tag="alpha")
                nc.scalar.activation(
                    out=alpha, in_=scm, func=ACTF.Exp, bias=float(b2val),
                )

                # ---- denominator: sum over all tokens ----
                tot = totps.tile([1, HT], F32, tag="tot")
                nc.tensor.matmul(out=tot, lhsT=ones_col, rhs=alpha,
                                 start=True, stop=True)
                tot_sb = finpool.tile([1, 1], F32, tag="tot_sb")
                nc.vector.tensor_reduce(
                    tot_sb, tot, axis=mybir.AxisListType.X, op=ALU.add,
                )
                inv = finpool.tile([1, 1], F32, tag="inv")
                nc.vector.reciprocal(inv, tot_sb)

                # ---- pass 2: acc[1, d] = sum_j alpha[:, j]^T @ V_j ----
                acc = accps.tile([1, D], F32, tag="acc")
                for j in range(HT):
                    nc.tensor.matmul(
                        out=acc,
                        lhsT=alpha[:, j:j + 1],
                        rhs=v3[:, j, :],
                        start=(j == 0),
                        stop=(j == HT - 1),
                    )
                nc.vector.tensor_scalar_mul(
                    oball[:, bi * D:(bi + 1) * D], acc, inv)

            nc.sync.dma_start(
                out=OUT.ap().rearrange("b d -> (b d)").rearrange("(o f) -> o f", o=1), in_=oball)

    nc.finalize()
    return nc


def _prep(K, V, mask, W, w1, b1, w2, b2):
    """Host-side input marshalling (no heavy compute)."""
    import ml_dtypes

    K = np.asarray(K, dtype=np.float32)
    V = np.asarray(V, dtype=np.float32)
    mask = np.asarray(mask)
    W = np.asarray(W, dtype=np.float32)
    w1 = np.asarray(w1, dtype=np.float32)
    b1 = np.asarray(b1, dtype=np.float32)
    w2 = np.asarray(w2, dtype=np.float32).reshape(-1)
    b2 = np.asarray(b2, dtype=np.float32).reshape(-1)

    Vb = np.ascontiguousarray(V.astype(np.float16))

    g = np.diagonal(W).astype(np.float32) * K          # [B, D]
    pos = w2 >= 0.0
    perm = np.argsort(~pos, kind="stable")             # positives first
    hp = int(pos.sum())
    wabs = (w1[:, perm] * np.abs(w2[perm])[None, :]).astype(np.float32)
    bias12 = (b1[perm] * np.abs(w2[perm])).astype(np.float32)
    has_bias = bool(np.any(bias12 != 0.0))

    # g arranged [B, 128, DC] so chunk c sits in column c (partition-major)
    gt = np.ascontiguousarray(g.reshape(B, DC, 128).transpose(0, 2, 1))
    # additive mask bias [B, 128, HT]: token j*128+p -> [p, j]
    mbias = np.where(mask, np.float32(MASK_FILL), np.float32(0.0)).astype(np.float32)
    mbias = np.ascontiguousarray(mbias.reshape(B, HT, 128).transpose(0, 2, 1))
    return Vb, gt, mbias, wabs, bias12, has_bias, hp, float(b2[0]) if b2.size else 0.0


def kernel(K, V, mask, W, w1, b1, w2, b2):
    from concourse import bass_utils

    Vb, gt, mbias, wabs, bias12, has_bias, hp, b2val = _prep(
        K, V, mask, W, w1, b1, w2, b2
    )
    nc = _build(hp, b2val, has_bias)

    in_maps = []
    for c in range(NCORES):
        sl = slice(c * BPC, (c + 1) * BPC)
        m = {
            "VB": Vb[sl],
            "GT": gt[sl],
            "MB": mbias[sl],
            "WA": wabs,
        }
        if has_bias:
            m["BI"] = bias12.reshape(1, HID)
        in_maps.append(m)

    res = bass_utils.run_bass_kernel_spmd(nc, in_maps, core_ids=list(range(NCORES)))
    out = np.concatenate([res.results[c]["OUT"] for c in range(NCORES)], axis=0)
    return out.astype(np.float32)



# revision 2
# speedup vs baseline: 1.3436x; 1.3436x over previous
"""TRN2 Bass kernel for nn_Attention_15590731285136.

Computation (per batch b):
    g      = diag(W) * K[b]                       # [d]
    score  = relu(V[b] @ (g[:,None]*w1) + b1) @ w2 + b2   # [h]
    score  = where(mask[b], MASK_FILL, score)
    alpha  = softmax(score)                        # over h
    out[b] = alpha @ V[b]                          # [d]

Sharding: data-parallel over batch, 8 batches per core on 8 NeuronCores.

Key transformations (v2):
  * The elementwise gate and w2's magnitudes fold into the weight matrix
    host-side: w12[b] = g[b] * (w1[:, perm] * |w2[perm]|), with a
    sign-grouping permutation (positive-w2 columns first).
  * The fc1 GEMM runs in fp8 (e4m3) with MatmulPerfMode.DoubleRow: each
    matmul contracts TWO 128-deep k-slices per pass, 2x the fp16 rate.
    w12 is scaled by S (power of two) to sit in e4m3's dynamic range;
    softmax is invariant up to the final exp(score/S) which folds 1/S
    into the activation's scale operand.
  * All device-side layouts (V^T fp8 for fc1's stationary operand,
    natural V fp16 for the alpha@V pass, gated w12 per batch, additive
    mask bias) are precomputed host-side, so every DMA is a plain
    contiguous row load - no DMA transposes, no on-device transposes,
    no on-device gating.
  * relu+rowsum of fc1 runs fused on ScalarE (ACT, positive-w2 group)
    and VectorE (DVE, negative group) via accum_out.
  * The softmax denominator + alpha@V of batch i are emitted after the
    fc1 loop of batch i+1 (software pipelining) so the PE never waits
    for alpha.
"""

import numpy as np

B, H, D, HID = 64, 2048, 512, 512
NCORES = 8
BPC = B // NCORES          # batches per core
HT = H // 128              # 16 token tiles per batch
DC = D // 128              # 4 contraction chunks
MASK_FILL = -2.0**32 + 1.0
PRE = 3                    # batches of loads in flight ahead of compute


def _build(hp, b2val, inv_s, has_bias):
    import concourse.mybir as mybir
    from concourse import bacc
    from concourse.tile import TileContext

    F32 = mybir.dt.float32
    F16 = mybir.dt.float16
    F8 = mybir.dt.float8e4
    ACTF = mybir.ActivationFunctionType
    ALU = mybir.AluOpType
    DR = mybir.MatmulPerfMode.DoubleRow

    nc = bacc.Bacc(trn_type="TRN2", num_devices=NCORES)

    # all inputs pre-arranged host-side into [128, cols] partition-major
    VT8 = nc.dram_tensor("VT8", (128, BPC * DC * H), F8, kind="ExternalInput")
    V16 = nc.dram_tensor("V16", (128, BPC * HT * D), F16, kind="ExternalInput")
    W12 = nc.dram_tensor("W12", (128, BPC * DC * HID), F8, kind="ExternalInput")
    MB = nc.dram_tensor("MB", (128, BPC * HT), F32, kind="ExternalInput")
    if has_bias:
        BI = nc.dram_tensor("BI", (1, HID), F16, kind="ExternalInput")
    OUT = nc.dram_tensor("OUT", (BPC, D), F32, kind="ExternalOutput")

    DCH = DC * H
    HTD = HT * D
    DCN = DC * HID

    with TileContext(nc) as tc:
        with (
            tc.tile_pool(name="const", bufs=1) as cpool,
            tc.tile_pool(name="vt", bufs=PRE + 1) as vtpool,
            tc.tile_pool(name="v", bufs=PRE + 2) as vpool,
            tc.tile_pool(name="w12", bufs=PRE + 1) as wpool,
            tc.tile_pool(name="scr", bufs=4) as scrpool,
            tc.tile_pool(name="small", bufs=8) as spool,
            tc.tile_pool(name="alpha", bufs=3) as alpool,
            tc.tile_pool(name="fin", bufs=4) as finpool,
            tc.tile_pool(name="fc1_ps", bufs=4, space="PSUM") as fc1ps,
            tc.tile_pool(name="tot_ps", bufs=2, space="PSUM") as totps,
            tc.tile_pool(name="acc_ps", bufs=2, space="PSUM") as accps,
        ):
            def emit_loads(bi):
                vt = vtpool.tile([128, DCH], F8, tag="vt")
                for c in range(DC):
                    nc.sync.dma_start(
                        out=vt[:, c * H:(c + 1) * H],
                        in_=VT8.ap()[:, bi * DCH + c * H: bi * DCH + (c + 1) * H],
                    )
                w12 = wpool.tile([128, DCN], F8, tag="w12")
                nc.sync.dma_start(
                    out=w12, in_=W12.ap()[:, bi * DCN:(bi + 1) * DCN])
                v = vpool.tile([128, HTD], F16, tag="v")
                for q in range(4):
                    nc.gpsimd.dma_start(
                        out=v[:, q * 4 * D:(q + 1) * 4 * D],
                        in_=V16.ap()[:, bi * HTD + q * 4 * D:
                                     bi * HTD + (q + 1) * 4 * D],
                    )
                return vt, v, w12

            pending = [emit_loads(bi) for bi in range(min(PRE, BPC))]

            # ---- one-time constants ----
            ones_col = cpool.tile([128, 1], F16, tag="ones")
            nc.vector.memset(ones_col, 1.0)
            mall = cpool.tile([128, BPC * HT], F32, tag="mall")
            nc.sync.dma_start(out=mall, in_=MB.ap())
            oball = cpool.tile([1, BPC * D], F32, tag="oball")
            if has_bias:
                ones_row = cpool.tile([1, 128], F16, tag="orr")
                nc.vector.memset(ones_row, 1.0)
                bias_sb = cpool.tile([1, HID], F16, tag="bias")
                nc.sync.dma_start(out=bias_sb, in_=BI.ap())

            def emit_tail(st):
                bi, alpha, v = st
                # denominator: sum over all tokens via PE + reduce
                tot = totps.tile([1, HT], F32, tag="tot")
                nc.tensor.matmul(out=tot, lhsT=ones_col, rhs=alpha,
                                 start=True, stop=True)
                tot_sb = finpool.tile([1, 1], F32, tag="tot_sb")
                nc.vector.tensor_reduce(
                    tot_sb, tot, axis=mybir.AxisListType.X, op=ALU.add)
                inv = finpool.tile([1, 1], F32, tag="inv")
                nc.vector.reciprocal(inv, tot_sb)
                # alpha @ V
                acc = accps.tile([1, D], F32, tag="acc")
                for j in range(HT):
                    nc.tensor.matmul(
                        out=acc,
                        lhsT=alpha[:, j:j + 1],
                        rhs=v[:, j * D:(j + 1) * D],
                        start=(j == 0),
                        stop=(j == HT - 1),
                    )
                nc.vector.tensor_scalar_mul(
                    oball[:, bi * D:(bi + 1) * D], acc, inv)

            deferred = None
            for bi in range(BPC):
                if bi + PRE < BPC:
                    pending.append(emit_loads(bi + PRE))
                vt, v, w12 = pending.pop(0)
                vt3 = vt.rearrange("p (c h) -> p c h", c=DC)
                w3 = w12.rearrange("p (c n) -> p c n", c=DC)
                mb = mall[:, bi * HT:(bi + 1) * HT]

                sp = spool.tile([128, HT], F32, tag="sp")
                sn = spool.tile([128, HT], F32, tag="sn")
                if hp == 0:
                    nc.vector.memset(sp, 0.0)
                if hp == HID:
                    nc.vector.memset(sn, 0.0)

                # ---- fc1 (fp8 DoubleRow) + fused relu/rowsum per tile ----
                for j in range(HT):
                    fc1 = fc1ps.tile([128, HID], F32, tag="fc1")
                    for pr in range(2):
                        nc.tensor.matmul(
                            out=fc1,
                            lhsT=vt3[:, 2 * pr:2 * pr + 2,
                                     j * 128:(j + 1) * 128],
                            rhs=w3[:, 2 * pr:2 * pr + 2, :],
                            start=(pr == 0),
                            stop=(pr == 1) and not has_bias,
                            perf_mode=DR,
                        )
                    if has_bias:
                        nc.tensor.matmul(
                            out=fc1, lhsT=ones_row, rhs=bias_sb,
                            start=False, stop=True,
                        )
                    if hp > 0:
                        scra = scrpool.tile([128, HID], F16, tag="scra")
                        nc.scalar.activation(
                            out=scra[:, :hp], in_=fc1[:, :hp], func=ACTF.Relu,
                            accum_out=sp[:, j:j + 1],
                        )
                    if hp < HID:
                        scrd = scrpool.tile([128, HID], F16, tag="scrd")
                        nc.vector.tensor_scalar(
                            out=scrd[:, hp:], in0=fc1[:, hp:],
                            scalar1=0.0, scalar2=None,
                            op0=ALU.max, op1=ALU.add,
                            accum_out=sn[:, j:j + 1],
                        )

                # ---- scores -> masked -> exp(score/S) ----
                sc = spool.tile([128, HT], F32, tag="sc")
                nc.vector.tensor_sub(sc, sp, sn)
                scm = spool.tile([128, HT], F32, tag="scm")
                nc.vector.tensor_add(scm, sc, mb)
                alpha = alpool.tile([128, HT], F16, tag="alpha")
                nc.scalar.activation(
                    out=alpha, in_=scm, func=ACTF.Exp,
                    bias=float(b2val), scale=float(inv_s),
                )

                if deferred is not None:
                    emit_tail(deferred)
                deferred = (bi, alpha, v)

            emit_tail(deferred)
            nc.sync.dma_start(
                out=OUT.ap().rearrange("b d -> (b d)")
                    .rearrange("(o f) -> o f", o=1),
                in_=oball)

    nc.finalize()
    return nc


def _prep(K, V, mask, W, w1, b1, w2, b2):
    """Host-side input marshalling (no device work)."""
    import ml_dtypes

    E4 = ml_dtypes.float8_e4m3   # TRN-style e4m3, max normal 240

    K = np.asarray(K, dtype=np.float32)
    V = np.asarray(V, dtype=np.float32)
    mask = np.asarray(mask)
    W = np.asarray(W, dtype=np.float32)
    w1 = np.asarray(w1, dtype=np.float32)
    b1 = np.asarray(b1, dtype=np.float32)
    w2 = np.asarray(w2, dtype=np.float32).reshape(-1)
    b2 = np.asarray(b2, dtype=np.float32).reshape(-1)

    g = np.diagonal(W).astype(np.float32)[None, :] * K       # [B, D]
    pos = w2 >= 0.0
    perm = np.argsort(~pos, kind="stable")                   # positives first
    hp = int(pos.sum())
    wabs = w1[:, perm] * np.abs(w2[perm])[None, :]           # [D, HID]
    bias12 = (b1[perm] * np.abs(w2[perm])).astype(np.float32)
    has_bias = bool(np.any(bias12 != 0.0))

    w12_all = g[:, :, None] * wabs[None, :, :]               # [B, D, HID]
    wmax = float(np.abs(w12_all).max()) + 1e-30
    s_exp = np.floor(np.log2(224.0 / wmax))
    if has_bias:
        bmax = float(np.abs(bias12).max()) + 1e-30
        s_exp = min(s_exp, np.floor(np.log2(3.0e4 / bmax)))
    S = float(2.0 ** s_exp)

    # fp8 gated+scaled weights: [B, 128, DC*HID], chunk c = d rows
    # [c*128, (c+1)*128)
    w12q = np.clip(w12_all * S, -240, 240).astype(E4)
    w12q = np.ascontiguousarray(
        w12q.reshape(B, DC, 128, HID).transpose(0, 2, 1, 3)
    ).reshape(B, 128, DC * HID)

    # fp8 V^T: [B, 128, DC*H]
    vt8 = np.clip(V, -240, 240).astype(E4).transpose(0, 2, 1)  # [B, D, H]
    vt8 = np.ascontiguousarray(
        vt8.reshape(B, DC, 128, H).transpose(0, 2, 1, 3)
    ).reshape(B, 128, DC * H)

    # fp16 natural V: [B, 128, HT*D], token tile j = tokens [j*128,(j+1)*128)
    v16 = np.ascontiguousarray(
        V.astype(np.float16).reshape(B, HT, 128, D).transpose(0, 2, 1, 3)
    ).reshape(B, 128, HT * D)

    # additive mask bias (pre-scaled by S): [B, 128, HT], token j*128+p -> [p, j]
    mbias = np.where(mask, np.float32(MASK_FILL * S), np.float32(0.0))
    mbias = np.ascontiguousarray(
        mbias.astype(np.float32).reshape(B, HT, 128).transpose(0, 2, 1))

    bias_sc = (bias12 * S).astype(np.float16)
    return (vt8, v16, w12q, mbias, bias_sc, has_bias, hp, 1.0 / S,
            float(b2[0]) if b2.size else 0.0)


def _core_maps(vt8, v16, w12q, mbias, bias_sc, has_bias):
    in_maps = []
    for c in range(NCORES):
        sl = slice(c * BPC, (c + 1) * BPC)
        m = {
            "VT8": np.ascontiguousarray(
                vt8[sl].transpose(1, 0, 2)).reshape(128, BPC * DC * H),
            "V16": np.ascontiguousarray(
                v16[sl].transpose(1, 0, 2)).reshape(128, BPC * HT * D),
            "W12": np.ascontiguousarray(
                w12q[sl].transpose(1, 0, 2)).reshape(128, BPC * DC * HID),
            "MB": np.ascontiguousarray(
                mbias[sl].transpose(1, 0, 2)).reshape(128, BPC * HT),
        }
        if has_bias:
            m["BI"] = bias_sc.reshape(1, HID)
        in_maps.append(m)
    return in_maps


def kernel(K, V, mask, W, w1, b1, w2, b2):
    from concourse import bass_utils

    vt8, v16, w12q, mbias, bias_sc, has_bias, hp, inv_s, b2val = _prep(
        K, V, mask, W, w1, b1, w2, b2
    )
    nc = _build(hp, b2val, inv_s, has_bias)
    in_maps = _core_maps(vt8, v16, w12q, mbias, bias_sc, has_bias)
    res = bass_utils.run_bass_kernel_spmd(nc, in_maps, core_ids=list(range(NCORES)))
    out = np.concatenate([res.results[c]["OUT"] for c in range(NCORES)], axis=0)
    return out.astype(np.float32)


# revision 3
# speedup vs baseline: 2.2732x; 1.6919x over previous
"""TRN2 Bass kernel for nn_Attention_15590731285136.

Computation (per batch b):
    g      = diag(W) * K[b]                       # [d]
    score  = relu(V[b] @ (g[:,None]*w1) + b1) @ w2 + b2   # [h]
    score  = where(mask[b], MASK_FILL, score)
    alpha  = softmax(score)                        # over h
    out[b] = alpha @ V[b]                          # [d]

Sharding: data-parallel over batch, 8 batches per core on 8 NeuronCores.

Key transformations (v3):
  * Token compaction: masked tokens have alpha == 0 exactly (their score
    is -2^32), so the host gathers only the unmasked tokens of each batch
    (~1024 of 2048) and pads to a multiple of 128. Padding tokens get the
    mask-fill score bias, so their alpha is exactly 0 too. This halves
    the fc1 GEMM, the relu/rowsum work, the alpha@V pass and the DMA
    traffic, with bit-identical math for the surviving tokens.
  * The elementwise gate and w2's magnitudes fold into the weight matrix
    host-side: w12[b] = g[b] * (w1[:, perm] * |w2[perm]|), with a
    sign-grouping permutation (positive-w2 columns first).
  * The fc1 GEMM runs in fp8 (e4m3) with MatmulPerfMode.DoubleRow: each
    matmul contracts TWO 128-deep k-slices per pass, 2x the fp16 rate.
    w12 is scaled by S (power of two) to sit in e4m3's dynamic range;
    softmax is invariant up to the final exp(score/S) which folds 1/S
    into the activation's scale operand.
  * All device-side layouts (compacted V^T fp8, compacted natural V fp16,
    gated w12, additive mask bias) are precomputed host-side, so every
    DMA is a plain contiguous row load.
  * relu+rowsum of fc1 runs fused on ScalarE (ACT, positive-w2 group)
    and VectorE (DVE, negative group) via accum_out; the first KSW token
    tiles' positive group also goes to DVE to balance the two engines.
  * The softmax denominator + alpha@V of batch i are emitted after the
    fc1 loop of batch i+1 (software pipelining) so the PE never waits
    for alpha.
"""

import numpy as np

B, H, D, HID = 64, 2048, 512, 512
NCORES = 8
BPC = B // NCORES          # batches per core
DC = D // 128              # 4 contraction chunks
MASK_FILL = -2.0**32 + 1.0
PRE = 3                    # batches of loads in flight ahead of compute
KSW = 2                    # leading token tiles whose pos-group runs on DVE


def _build(hp, b2val, inv_s, has_bias, htp):
    import concourse.mybir as mybir
    from concourse import bacc
    from concourse.tile import TileContext

    F32 = mybir.dt.float32
    F16 = mybir.dt.float16
    F8 = mybir.dt.float8e4
    ACTF = mybir.ActivationFunctionType
    ALU = mybir.AluOpType
    DR = mybir.MatmulPerfMode.DoubleRow

    HP = htp * 128             # padded token count
    DCH = DC * HP
    HTD = htp * D
    DCN = DC * HID

    nc = bacc.Bacc(trn_type="TRN2", num_devices=NCORES)

    # all inputs pre-arranged host-side into [128, cols] partition-major
    VT8 = nc.dram_tensor("VT8", (128, BPC * DCH), F8, kind="ExternalInput")
    V16 = nc.dram_tensor("V16", (128, BPC * HTD), F16, kind="ExternalInput")
    W12 = nc.dram_tensor("W12", (128, BPC * DCN), F8, kind="ExternalInput")
    MB = nc.dram_tensor("MB", (128, BPC * htp), F32, kind="ExternalInput")
    if has_bias:
        BI = nc.dram_tensor("BI", (1, HID), F16, kind="ExternalInput")
    OUT = nc.dram_tensor("OUT", (BPC, D), F32, kind="ExternalOutput")

    with TileContext(nc) as tc:
        with (
            tc.tile_pool(name="const", bufs=1) as cpool,
            tc.tile_pool(name="vt", bufs=PRE + 1) as vtpool,
            tc.tile_pool(name="v", bufs=PRE + 2) as vpool,
            tc.tile_pool(name="w12", bufs=PRE + 1) as wpool,
            tc.tile_pool(name="scr", bufs=4) as scrpool,
            tc.tile_pool(name="small", bufs=8) as spool,
            tc.tile_pool(name="alpha", bufs=3) as alpool,
            tc.tile_pool(name="fin", bufs=4) as finpool,
            tc.tile_pool(name="fc1_ps", bufs=4, space="PSUM") as fc1ps,
            tc.tile_pool(name="tot_ps", bufs=2, space="PSUM") as totps,
            tc.tile_pool(name="acc_ps", bufs=2, space="PSUM") as accps,
        ):
            def emit_w(bi):
                w12 = wpool.tile([128, DCN], F8, tag="w12")
                nc.sync.dma_start(
                    out=w12, in_=W12.ap()[:, bi * DCN:(bi + 1) * DCN])
                vt = vtpool.tile([128, DCH], F8, tag="vt")
                nc.sync.dma_start(
                    out=vt, in_=VT8.ap()[:, bi * DCH:(bi + 1) * DCH])
                return vt, w12

            def emit_v(bi):
                v = vpool.tile([128, HTD], F16, tag="v")
                nc.gpsimd.dma_start(
                    out=v, in_=V16.ap()[:, bi * HTD:(bi + 1) * HTD])
                return v

            pend_w = [emit_w(bi) for bi in range(min(PRE, BPC))]
            pend_v = [emit_v(bi) for bi in range(min(PRE - 1, BPC))]

            # ---- one-time constants ----
            ones_col = cpool.tile([128, 1], F16, tag="ones")
            nc.vector.memset(ones_col, 1.0)
            mall = cpool.tile([128, BPC * htp], F32, tag="mall")
            nc.sync.dma_start(out=mall, in_=MB.ap())
            oball = cpool.tile([1, BPC * D], F32, tag="oball")
            if has_bias:
                ones_row = cpool.tile([1, 128], F16, tag="orr")
                nc.vector.memset(ones_row, 1.0)
                bias_sb = cpool.tile([1, HID], F16, tag="bias")
                nc.sync.dma_start(out=bias_sb, in_=BI.ap())

            def emit_tail(st):
                bi, alpha, v = st
                # denominator: sum over all tokens via PE + reduce
                tot = totps.tile([1, htp], F32, tag="tot")
                nc.tensor.matmul(out=tot, lhsT=ones_col, rhs=alpha,
                                 start=True, stop=True)
                tot_sb = finpool.tile([1, 1], F32, tag="tot_sb")
                nc.vector.tensor_reduce(
                    tot_sb, tot, axis=mybir.AxisListType.X, op=ALU.add)
                inv = finpool.tile([1, 1], F32, tag="inv")
                nc.vector.reciprocal(inv, tot_sb)
                # alpha @ V
                acc = accps.tile([1, D], F32, tag="acc")
                for j in range(htp):
                    nc.tensor.matmul(
                        out=acc,
                        lhsT=alpha[:, j:j + 1],
                        rhs=v[:, j * D:(j + 1) * D],
                        start=(j == 0),
                        stop=(j == htp - 1),
                    )
                nc.vector.tensor_scalar_mul(
                    oball[:, bi * D:(bi + 1) * D], acc, inv)

            deferred = None
            for bi in range(BPC):
                if bi + PRE < BPC:
                    pend_w.append(emit_w(bi + PRE))
                if bi + PRE - 1 < BPC:
                    pend_v.append(emit_v(bi + PRE - 1))
                vt, w12 = pend_w.pop(0)
                v = pend_v.pop(0)
                vt3 = vt.rearrange("p (c h) -> p c h", c=DC)
                w3 = w12.rearrange("p (c n) -> p c n", c=DC)
                mb = mall[:, bi * htp:(bi + 1) * htp]

                sp = spool.tile([128, htp], F32, tag="sp")
                sn = spool.tile([128, htp], F32, tag="sn")
                if hp == 0:
                    nc.vector.memset(sp, 0.0)
                if hp == HID:
                    nc.vector.memset(sn, 0.0)

                # ---- fc1 (fp8 DoubleRow) + fused relu/rowsum per tile ----
                for j in range(htp):
                    fc1 = fc1ps.tile([128, HID], F32, tag="fc1")
                    for pr in range(2):
                        nc.tensor.matmul(
                            out=fc1,
                            lhsT=vt3[:, 2 * pr:2 * pr + 2,
                                     j * 128:(j + 1) * 128],
                            rhs=w3[:, 2 * pr:2 * pr + 2, :],
                            start=(pr == 0),
                            stop=(pr == 1) and not has_bias,
                            perf_mode=DR,
                        )
                    if has_bias:
                        nc.tensor.matmul(
                            out=fc1, lhsT=ones_row, rhs=bias_sb,
                            start=False, stop=True,
                        )
                    if hp > 0:
                        if j < KSW:
                            scrp = scrpool.tile([128, HID], F16, tag="scrp")
                            nc.vector.tensor_scalar(
                                out=scrp[:, :hp], in0=fc1[:, :hp],
                                scalar1=0.0, scalar2=None,
                                op0=ALU.max, op1=ALU.add,
                                accum_out=sp[:, j:j + 1],
                            )
                        else:
                            scra = scrpool.tile([128, HID], F16, tag="scra")
                            nc.scalar.activation(
                                out=scra[:, :hp], in_=fc1[:, :hp],
                                func=ACTF.Relu,
                                accum_out=sp[:, j:j + 1],
                            )
                    if hp < HID:
                        scrd = scrpool.tile([128, HID], F16, tag="scrd")
                        nc.vector.tensor_scalar(
                            out=scrd[:, hp:], in0=fc1[:, hp:],
                            scalar1=0.0, scalar2=None,
                            op0=ALU.max, op1=ALU.add,
                            accum_out=sn[:, j:j + 1],
                        )

                # ---- scores -> masked -> exp(score/S) ----
                sc = spool.tile([128, htp], F32, tag="sc")
                nc.vector.tensor_sub(sc, sp, sn)
                scm = spool.tile([128, htp], F32, tag="scm")
                nc.vector.tensor_add(scm, sc, mb)
                alpha = alpool.tile([128, htp], F16, tag="alpha")
                nc.scalar.activation(
                    out=alpha, in_=scm, func=ACTF.Exp,
                    bias=float(b2val), scale=float(inv_s),
                )

                if deferred is not None:
                    emit_tail(deferred)
                deferred = (bi, alpha, v)

            emit_tail(deferred)
            nc.sync.dma_start(
                out=OUT.ap().rearrange("b d -> (b d)")
                    .rearrange("(o f) -> o f", o=1),
                in_=oball)

    nc.finalize()
    return nc


def _prep(K, V, mask, W, w1, b1, w2, b2):
    """Host-side input marshalling (no device work)."""
    import ml_dtypes

    E4 = ml_dtypes.float8_e4m3   # TRN-style e4m3, max normal 240

    K = np.asarray(K, dtype=np.float32)
    V = np.asarray(V, dtype=np.float32)
    mask = np.asarray(mask).astype(bool)
    W = np.asarray(W, dtype=np.float32)
    w1 = np.asarray(w1, dtype=np.float32)
    b1 = np.asarray(b1, dtype=np.float32)
    w2 = np.asarray(w2, dtype=np.float32).reshape(-1)
    b2 = np.asarray(b2, dtype=np.float32).reshape(-1)

    g = np.diagonal(W).astype(np.float32)[None, :] * K       # [B, D]
    pos = w2 >= 0.0
    perm = np.argsort(~pos, kind="stable")                   # positives first
    hp = int(pos.sum())
    wabs = w1[:, perm] * np.abs(w2[perm])[None, :]           # [D, HID]
    bias12 = (b1[perm] * np.abs(w2[perm])).astype(np.float32)
    has_bias = bool(np.any(bias12 != 0.0))

    w12_all = g[:, :, None] * wabs[None, :, :]               # [B, D, HID]
    wmax = float(np.abs(w12_all).max()) + 1e-30
    s_exp = np.floor(np.log2(224.0 / wmax))
    if has_bias:
        bmax = float(np.abs(bias12).max()) + 1e-30
        s_exp = min(s_exp, np.floor(np.log2(3.0e4 / bmax)))
    S = float(2.0 ** s_exp)

    # ---- token compaction: keep only unmasked tokens, pad to mult of 128
    cnt = (~mask).sum(1)
    HP = max(128, int(np.ceil(cnt.max() / 128.0)) * 128)
    htp = HP // 128
    Vc = np.zeros((B, HP, D), dtype=np.float32)
    mbias = np.full((B, HP), np.float32(MASK_FILL * S), dtype=np.float32)
    for b in range(B):
        idx = np.nonzero(~mask[b])[0]
        Vc[b, :len(idx)] = V[b, idx]
        mbias[b, :len(idx)] = 0.0

    # fp8 gated+scaled weights: [B, 128, DC*HID], chunk c = d rows
    # [c*128, (c+1)*128)
    w12q = np.clip(w12_all * S, -240, 240).astype(E4)
    w12q = np.ascontiguousarray(
        w12q.reshape(B, DC, 128, HID).transpose(0, 2, 1, 3)
    ).reshape(B, 128, DC * HID)

    # fp8 V^T (compacted): [B, 128, DC*HP]
    vt8 = np.clip(Vc, -240, 240).astype(E4).transpose(0, 2, 1)  # [B, D, HP]
    vt8 = np.ascontiguousarray(
        vt8.reshape(B, DC, 128, HP).transpose(0, 2, 1, 3)
    ).reshape(B, 128, DC * HP)

    # fp16 natural V (compacted): [B, 128, htp*D]
    v16 = np.ascontiguousarray(
        Vc.astype(np.float16).reshape(B, htp, 128, D).transpose(0, 2, 1, 3)
    ).reshape(B, 128, htp * D)

    # additive mask bias (pre-scaled by S): [B, 128, htp]
    mbias = np.ascontiguousarray(
        mbias.reshape(B, htp, 128).transpose(0, 2, 1))

    bias_sc = (bias12 * S).astype(np.float16)
    return (vt8, v16, w12q, mbias, bias_sc, has_bias, hp, 1.0 / S,
            float(b2[0]) if b2.size else 0.0, htp)


def _core_maps(vt8, v16, w12q, mbias, bias_sc, has_bias, htp):
    HP = htp * 128
    in_maps = []
    for c in range(NCORES):
        sl = slice(c * BPC, (c + 1) * BPC)
        m = {
            "VT8": np.ascontiguousarray(
                vt8[sl].transpose(1, 0, 2)).reshape(128, BPC * DC * HP),
            "V16": np.ascontiguousarray(
                v16[sl].transpose(1, 0, 2)).reshape(128, BPC * htp * D),
            "W12": np.ascontiguousarray(
                w12q[sl].transpose(1, 0, 2)).reshape(128, BPC * DC * HID),
            "MB": np.ascontiguousarray(
                mbias[sl].transpose(1, 0, 2)).reshape(128, BPC * htp),
        }
        if has_bias:
            m["BI"] = bias_sc.reshape(1, HID)
        in_maps.append(m)
    return in_maps


def kernel(K, V, mask, W, w1, b1, w2, b2):
    from concourse import bass_utils

    vt8, v16, w12q, mbias, bias_sc, has_bias, hp, inv_s, b2val, htp = _prep(
        K, V, mask, W, w1, b1, w2, b2
    )
    nc = _build(hp, b2val, inv_s, has_bias, htp)
    in_maps = _core_maps(vt8, v16, w12q, mbias, bias_sc, has_bias, htp)
    res = bass_utils.run_bass_kernel_spmd(nc, in_maps, core_ids=list(range(NCORES)))
    out = np.concatenate([res.results[c]["OUT"] for c in range(NCORES)], axis=0)
    return out.astype(np.float32)
